# revision 1
# baseline (speedup 1.0000x reference)
"""Sparse (2D local-window) attention on 8 TRN2 NeuronCores.

Strategy: pure data-parallel over batch (32 batches -> 4 per core), no
collectives. Per core, per batch:
  1. xT [C, N] loaded directly via DMA x-bar transpose (x pre-cast bf16).
  2. qkv projection as qT/kT [dims, tok] and V token-major [tok, dims]
     (attention scale folded into the Q columns of w_qkv host-side).
  3. Attention per (head, query-row h): S^T = K_row_i @ Q_row_h^T for the
     <=7 visible key rows, plus an accumulated identity@negband matmul that
     adds -30 outside the |dj|<=5 band (additive mask); exp on ScalarE;
     column-sum via ones-matmul; PV accumulate O^T [64, q]; multiply by the
     matmul-replicated reciprocal denominator.
  4. proj: final[tok, C] = O^T-chunks.T @ w_proj + bias (bias via K=1 matmul
     accumulate), f32 out.
"""

import numpy as np
import ml_dtypes

import concourse.bass as bass
import concourse.mybir as mybir
import concourse.tile as _tilemod
from concourse.tile import TileContext
from concourse.bass_utils import run_bass_kernel_spmd
from concourse.masks import make_identity


def _split_drain_and_barrier(self, tick_clock, wait_clock):
    # The pinned walrus accepts at most one sync-wait per instruction; the
    # stock kernel-tail drain carries one wait per logical proc (~12). Split
    # it into a chain of single-wait drains on the SP queue.
    nc = self.nc
    drain_inst = nc.sync.drain()
    wait_clock.add_sem_waits(
        drain_inst.ins, _tilemod.ScopedClock({None: tick_clock.global_clock})
    )
    si = drain_inst.ins.sync_info
    if si is not None and len(si.on_wait) > 1:
        waits = list(si.on_wait)
        upd = list(si.on_update)
        drain_inst.ins.sync_info = mybir.SyncInfo(on_wait=[waits[0]], on_update=[])
        for w in waits[1:-1]:
            d2 = nc.sync.drain()
            d2.ins.sync_info = mybir.SyncInfo(on_wait=[w], on_update=[])
        dl = nc.sync.drain()
        dl.ins.sync_info = mybir.SyncInfo(on_wait=[waits[-1]], on_update=upd)
    nc.all_engine_barrier()
    popped = nc._tile_sem_poison_stack.pop()
    assert popped is self._sem_poison
    nc.clear_and_free_semaphores(list(self.sems.allocated().values()))
    nc.all_engine_barrier()


_tilemod.TileContext._drain_and_barrier = _split_drain_and_barrier

B, H, W, C, NUM_HEADS = 32, 8, 128, 512, 8
HK, WK = 7, 11
N = H * W
HEAD_DIM = C // NUM_HEADS
SCALE = HEAD_DIM ** (-0.5)
NCORES = 8
BPC = B // NCORES
NCH = C // 128  # 4 contraction chunks

F32 = mybir.dt.float32
BF16 = mybir.dt.bfloat16
AF = mybir.ActivationFunctionType


def build_nc():
    nc = bass.Bass()
    x = nc.dram_tensor("x", [BPC, N, C], BF16, kind="ExternalInput")
    wqkv = nc.dram_tensor("w_qkv", [C, 3 * C], BF16, kind="ExternalInput")
    wproj = nc.dram_tensor("w_proj", [C, C], BF16, kind="ExternalInput")
    bproj = nc.dram_tensor("b_proj", [1, C], BF16, kind="ExternalInput")
    negband = nc.dram_tensor("negband", [W, W], BF16, kind="ExternalInput")
    out = nc.dram_tensor("out", [BPC, N, C], F32, kind="ExternalOutput")

    with TileContext(nc) as tc:
        with (
            tc.tile_pool(name="const", bufs=1) as constp,
            tc.tile_pool(name="xin", bufs=32) as xinp,
            tc.tile_pool(name="xt", bufs=2) as xtp,
            tc.tile_pool(name="qkv", bufs=2) as qkvp,
            tc.tile_pool(name="pbuf", bufs=8) as pp,
            tc.tile_pool(name="ot", bufs=2) as otp,
            tc.tile_pool(name="osb", bufs=4) as osbp,
            tc.tile_pool(name="small", bufs=4) as smallp,
            tc.tile_pool(name="psmm", bufs=2, space="PSUM") as psmm,
            tc.tile_pool(name="pss", bufs=2, space="PSUM") as pss,
            tc.tile_pool(name="pso", bufs=3, space="PSUM") as pso,
            tc.tile_pool(name="psd", bufs=1, space="PSUM") as psd,
        ):
            ident = constp.tile([128, 128], BF16, tag="ident")
            make_identity(nc, ident)
            ones = constp.tile([128, 128], BF16, tag="ones")
            nc.gpsimd.memset(ones, 1.0)
            nb_sb = constp.tile([W, W], BF16, tag="nb")
            nc.sync.dma_start(out=nb_sb, in_=negband[:, :])
            wqkv_sb = []
            for c in range(NCH):
                t = constp.tile([128, 3 * C], BF16, tag=f"wqkv{c}", name=f"wqkv{c}")
                nc.sync.dma_start(out=t, in_=wqkv[c * 128:(c + 1) * 128, :])
                wqkv_sb.append(t)
            wproj_sb = []
            for c in range(NCH):
                t = constp.tile([128, C], BF16, tag=f"wproj{c}", name=f"wproj{c}")
                nc.sync.dma_start(out=t, in_=wproj[c * 128:(c + 1) * 128, :])
                wproj_sb.append(t)
            bias_sb = constp.tile([1, C], BF16, tag="bias")
            nc.sync.dma_start(out=bias_sb, in_=bproj[:, :])

            for b in range(BPC):
                # ---- x load + PE transpose to xT[c][:, tok] ----
                xT = [xtp.tile([128, N], BF16, tag=f"xt{c}", name=f"xt{c}") for c in range(NCH)]
                for t in range(8):
                    xin = xinp.tile([128, C], BF16, tag="xin")
                    nc.sync.dma_start(out=xin, in_=x[b, t * 128:(t + 1) * 128, :])
                    for c in range(NCH):
                        tps = pss.tile([128, 128], BF16, tag="s")
                        nc.tensor.transpose(tps, xin[:, c * 128:(c + 1) * 128], ident)
                        nc.vector.tensor_copy(xT[c][:, t * 128:(t + 1) * 128], tps)

                # ---- qT / kT [dims, tok] ----
                qT = [qkvp.tile([128, N], BF16, tag=f"q{m}", name=f"q{m}") for m in range(4)]
                kT = [qkvp.tile([128, N], BF16, tag=f"k{m}", name=f"k{m}") for m in range(4)]
                for m in range(4):
                    for dst, base in ((qT, 0), (kT, C)):
                        for nh in range(2):
                            ps = psmm.tile([128, 512], F32, tag="mm")
                            for c in range(NCH):
                                nc.tensor.matmul(
                                    ps,
                                    wqkv_sb[c][:, base + m * 128: base + (m + 1) * 128],
                                    xT[c][:, nh * 512:(nh + 1) * 512],
                                    start=(c == 0),
                                    stop=(c == NCH - 1),
                                )
                            nc.vector.tensor_copy(dst[m][:, nh * 512:(nh + 1) * 512], ps)
                # ---- V token-major [tok, dims] ----
                v_sb = [qkvp.tile([128, C], BF16, tag=f"v{t}", name=f"v{t}") for t in range(8)]
                for t in range(8):
                    ps = psmm.tile([128, 512], F32, tag="mm")
                    for c in range(NCH):
                        nc.tensor.matmul(
                            ps,
                            xT[c][:, t * 128:(t + 1) * 128],
                            wqkv_sb[c][:, 2 * C:3 * C],
                            start=(c == 0),
                            stop=(c == NCH - 1),
                        )
                    nc.vector.tensor_copy(v_sb[t], ps)

                # ---- attention ----
                oT = [otp.tile([128, N], BF16, tag=f"ot{cc}", name=f"ot{cc}") for cc in range(4)]
                for hd in range(NUM_HEADS):
                    qt = qT[hd // 2]
                    kt = kT[hd // 2]
                    pb = 64 * (hd % 2)
                    for h in range(H):
                        i0, i1 = max(0, h - 3), min(7, h + 3)
                        nblk = i1 - i0 + 1
                        p_sb = pp.tile([128, 7 * 128], BF16, tag="p")
                        for bi, i in enumerate(range(i0, i1 + 1)):
                            ps = pss.tile([128, 128], F32, tag="s")
                            nc.tensor.matmul(
                                ps,
                                kt[pb:pb + 64, i * 128:(i + 1) * 128],
                                qt[pb:pb + 64, h * 128:(h + 1) * 128],
                                start=True,
                                stop=False,
                            )
                            nc.tensor.matmul(
                                ps, ident, nb_sb, start=False, stop=True
                            )
                            nc.scalar.activation(
                                p_sb[:, bi * 128:(bi + 1) * 128], ps, AF.Exp
                            )
                        dps = psd.tile([1, 128], F32, tag="d")
                        for bi in range(nblk):
                            nc.tensor.matmul(
                                dps,
                                ones[:, 0:1],
                                p_sb[:, bi * 128:(bi + 1) * 128],
                                start=(bi == 0),
                                stop=(bi == nblk - 1),
                            )
                        rd32 = smallp.tile([1, 128], F32, tag="rd32")
                        nc.vector.reciprocal(rd32, dps)
                        rd = smallp.tile([1, 128], BF16, tag="rd")
                        nc.vector.tensor_copy(rd, rd32)
                        ops_ = pso.tile([64, 128], F32, tag="o")
                        for bi, i in enumerate(range(i0, i1 + 1)):
                            nc.tensor.matmul(
                                ops_,
                                v_sb[i][:, hd * 64:(hd + 1) * 64],
                                p_sb[:, bi * 128:(bi + 1) * 128],
                                start=(bi == 0),
                                stop=(bi == nblk - 1),
                            )
                        rps = pso.tile([64, 128], F32, tag="o")
                        nc.tensor.matmul(rps, ones[0:1, 0:64], rd, start=True, stop=True)
                        rsb = smallp.tile([64, 128], F32, tag="rsb")
                        nc.vector.tensor_copy(rsb, rps)
                        nc.vector.tensor_mul(
                            oT[hd // 2][pb:pb + 64, h * 128:(h + 1) * 128], ops_, rsb
                        )
                        fence = smallp.tile([1, 7 * 128], BF16, tag="fence")
                        nc.scalar.copy(fence[:, 0:nblk * 128], p_sb[0:1, 0:nblk * 128])

                # ---- proj + bias ----
                osb = osbp.tile([128, 8 * C], BF16, tag="osb")
                for t in range(8):
                    ps = psmm.tile([128, 512], F32, tag="mm")
                    for c in range(NCH):
                        nc.tensor.matmul(
                            ps,
                            oT[c][:, t * 128:(t + 1) * 128],
                            wproj_sb[c],
                            start=(c == 0),
                            stop=False,
                        )
                    nc.tensor.matmul(ps, ones[0:1, :], bias_sb, start=False, stop=True)
                    nc.vector.tensor_copy(osb[:, t * C:(t + 1) * C], ps)
                # one store per batch: DRAM viewed [t, p, c] -> partition p
                out_view = out[b].rearrange("(t p) c -> p t c", p=128)
                nc.gpsimd.dma_start(out=out_view, in_=osb.rearrange("p (t c) -> p t c", c=C))
    return nc


_NC_CACHE = None


def _get_nc():
    global _NC_CACHE
    if _NC_CACHE is None:
        _NC_CACHE = build_nc()
    return _NC_CACHE


def make_in_maps(inputs):
    x = np.asarray(inputs["x"], np.float32)
    w_qkv = np.asarray(inputs["w_qkv"], np.float32).copy()
    w_proj = np.asarray(inputs["w_proj"], np.float32)
    b_proj = np.asarray(inputs["b_proj"], np.float32)
    w_qkv[:, :C] *= SCALE
    bf = ml_dtypes.bfloat16
    inband = np.abs(np.arange(W)[:, None] - np.arange(W)[None, :]) <= WK // 2
    negband = np.where(inband, 0.0, -30.0).astype(bf)
    in_maps = []
    for core in range(NCORES):
        in_maps.append(
            {
                "x": np.ascontiguousarray(x[core * BPC:(core + 1) * BPC]).astype(bf),
                "w_qkv": w_qkv.astype(bf),
                "w_proj": w_proj.astype(bf),
                "b_proj": b_proj.reshape(1, C).astype(bf),
                "negband": negband,
            }
        )
    return in_maps


def kernel(**inputs):
    nc = _get_nc()
    in_maps = make_in_maps(inputs)
    res = run_bass_kernel_spmd(nc, in_maps, core_ids=list(range(NCORES)))
    outs = [np.asarray(res.results[i]["out"], np.float32) for i in range(NCORES)]
    return np.concatenate(outs, axis=0)



# revision 2
# speedup vs baseline: 112.3018x; 112.3018x over previous
"""Sparse (2D local-window) attention on 8 TRN2 NeuronCores — v2.

Data-parallel over batch (32 -> 4 per core), no collectives. Redesign vs v1:
  * All matmuls stream 512-wide (PSUM-bank-width) moving operands.
  * Attention blocked per (head, key-row i): S^T [128 keys, 1024 queries]
    computed only over the valid |h_q - i| <= 3 column range (band in H),
    -30 outside the |dj|<=5 W-band added via one identity@negband matmul
    accumulate per segment.
  * exp on ScalarE straight from PSUM into bf16 SBUF P tiles.
  * Softmax denominator comes free from the PV matmul: V is stored per head
    as 65 columns (64 dims + a ones column), so PV accumulation yields
    O^T rows 0..63 and the denominator in row 64.
  * Normalization: reciprocal (DVE) -> partition_broadcast (GpSimd) ->
    tensor_mul (DVE).
  * One-stage software pipeline over heads so ScalarE exp latency never
    stalls the PE matmul stream.
"""

import numpy as np
import ml_dtypes

import concourse.bass as bass
import concourse.mybir as mybir
import concourse.tile as _tilemod
from concourse.tile import TileContext
from concourse.bass_utils import run_bass_kernel_spmd
from concourse.masks import make_identity


def _split_drain_and_barrier(self, tick_clock, wait_clock):
    # The pinned walrus accepts at most one sync-wait per instruction; the
    # stock kernel-tail drain carries one wait per logical proc (~12). Split
    # it into a chain of single-wait drains on the SP queue.
    nc = self.nc
    drain_inst = nc.sync.drain()
    wait_clock.add_sem_waits(
        drain_inst.ins, _tilemod.ScopedClock({None: tick_clock.global_clock})
    )
    si = drain_inst.ins.sync_info
    if si is not None and len(si.on_wait) > 1:
        waits = list(si.on_wait)
        upd = list(si.on_update)
        drain_inst.ins.sync_info = mybir.SyncInfo(on_wait=[waits[0]], on_update=[])
        for w in waits[1:-1]:
            d2 = nc.sync.drain()
            d2.ins.sync_info = mybir.SyncInfo(on_wait=[w], on_update=[])
        dl = nc.sync.drain()
        dl.ins.sync_info = mybir.SyncInfo(on_wait=[waits[-1]], on_update=upd)
    nc.all_engine_barrier()
    popped = nc._tile_sem_poison_stack.pop()
    assert popped is self._sem_poison
    nc.clear_and_free_semaphores(list(self.sems.allocated().values()))
    nc.all_engine_barrier()


_tilemod.TileContext._drain_and_barrier = _split_drain_and_barrier


def _fix_multiwaits(nc):
    """The pinned walrus accepts at most one sync-wait per instruction.
    The tile wait-assigner can emit two (a cross-engine RAW wait plus a
    same-engine/queue WAW wait). Redistribute: move extra waits onto an
    earlier same-engine instruction with no wait. Safe iff the moved
    wait's producer (the wait_value-th updater of that semaphore)
    precedes the carrier in program order — then the carrier's new wait
    can always be satisfied.
    """
    _SELF_SEM = {
        "EngineType.PE": "PE_",
        "EngineType.Activation": "Activation_",
        "EngineType.DVE": "DVE_",
        "EngineType.Pool": "Pool_",
        "EngineType.SP": "SP_",
    }

    moved = dropped = 0
    blocks = [blk for f in nc.m.functions for blk in f.blocks]
    for blk in blocks:
        blkins = list(blk.instructions)
        # position of each semaphore's k-th update within this block (DMA
        # queue sems update implicitly in hardware and never appear here —
        # waits on those are treated as unmovable)
        upd_pos = {}
        sem_count = {}
        for pos, ins in enumerate(blkins):
            si = ins.sync_info
            if si is None:
                continue
            for u in si.on_update:
                k = sem_count.get(u.ant_name, 0) + 1
                sem_count[u.ant_name] = k
                upd_pos[(u.ant_name, k)] = pos

        def producer_pos(w):
            return upd_pos.get((w.ant_name, w.wait_value))

        for pos, ins in enumerate(blkins):
            si = ins.sync_info
            if si is None or len(si.on_wait) <= 1:
                continue
            waits = list(si.on_wait)
            while len(waits) > 1:
                movable = [w for w in waits if producer_pos(w) is not None]
                movable.sort(key=producer_pos)
                movedone = False
                for w in movable:
                    ppos = producer_pos(w)
                    carrier = None
                    j = pos - 1
                    while j > ppos:
                        cand = blkins[j]
                        if cand.engine == ins.engine:
                            csi = cand.sync_info
                            if csi is None or not csi.on_wait:
                                carrier = cand
                                break
                        j -= 1
                    if carrier is not None:
                        csi = carrier.sync_info
                        upd = list(csi.on_update) if csi is not None else []
                        carrier.sync_info = mybir.SyncInfo(
                            on_wait=[w], on_update=upd
                        )
                        waits.remove(w)
                        moved += 1
                        movedone = True
                        break
                if movedone:
                    continue
                # all-DMA-ring-wait instructions: the immediately preceding
                # same-engine instruction is a safe carrier (queue issuers
                # are same-engine, so no packets counted by the wait can be
                # issued between an ADJACENT carrier and the instruction)
                if all(w.ant_name.startswith(("DMAHW", "DMASW")) for w in waits):
                    j = pos - 1
                    adj = None
                    while j >= 0:
                        if blkins[j].engine == ins.engine:
                            adj = blkins[j]
                            break
                        j -= 1
                    if adj is not None:
                        asi = adj.sync_info
                        if asi is None or not asi.on_wait:
                            upd = list(asi.on_update) if asi is not None else []
                            w = waits.pop(0)
                            adj.sync_info = mybir.SyncInfo(
                                on_wait=[w], on_update=upd
                            )
                            moved += 1
                            continue
                selfp = _SELF_SEM.get(str(ins.engine), "\x00")
                selfw = [w for w in waits if w.ant_name.startswith(selfp)]
                if selfw:
                    waits.remove(selfw[0])
                    dropped += 1
                else:
                    raise AssertionError(
                        f"cannot fix multi-wait on {ins.name}: "
                        + ",".join(w.ant_name for w in waits)
                    )
            ins.sync_info = mybir.SyncInfo(
                on_wait=waits, on_update=list(si.on_update)
            )
    return moved, dropped

B, H, W, C, NUM_HEADS = 32, 8, 128, 512, 8
HK, WK = 7, 11
N = H * W
HEAD_DIM = C // NUM_HEADS
SCALE = HEAD_DIM ** (-0.5)
NCORES = 8
BPC = B // NCORES
NCH = C // 128  # 4 contraction chunks

F32 = mybir.dt.float32
BF16 = mybir.dt.bfloat16
AF = mybir.ActivationFunctionType


def build_nc():
    nc = bass.Bass()
    x = nc.dram_tensor("x", [BPC, N, C], BF16, kind="ExternalInput")
    wqkv = nc.dram_tensor("w_qkv", [C, 3 * C], BF16, kind="ExternalInput")
    wproj = nc.dram_tensor("w_proj", [C, C], BF16, kind="ExternalInput")
    bproj = nc.dram_tensor("b_proj", [1, C], BF16, kind="ExternalInput")
    negband4 = nc.dram_tensor("negband4", [W, 4 * W], BF16, kind="ExternalInput")
    out = nc.dram_tensor("out", [BPC, N, C], F32, kind="ExternalOutput")
    scr = nc.dram_tensor("scr", [1, 32], BF16, kind="Internal")

    with TileContext(nc) as tc:
        with (
            tc.tile_pool(name="const", bufs=1) as constp,
            tc.tile_pool(name="xin", bufs=32) as xinp,
            tc.tile_pool(name="xt", bufs=2) as xtp,
            tc.tile_pool(name="qk", bufs=2) as qkp,
            tc.tile_pool(name="pp", bufs=2) as pp,
            tc.tile_pool(name="ot", bufs=2) as otp,
            tc.tile_pool(name="osb", bufs=2) as osbp,
            tc.tile_pool(name="small", bufs=2) as smallp,
            tc.tile_pool(name="ps", bufs=3, space="PSUM") as psp,
            tc.tile_pool(name="pstr", bufs=1, space="PSUM") as pstr,
            tc.tile_pool(name="pspv", bufs=3, space="PSUM") as pspv,
            tc.tile_pool(name="psbc", bufs=1, space="PSUM") as psbc,
        ):
            ident = constp.tile([128, 128], BF16, tag="ident", name="ident")
            make_identity(nc, ident)
            ones_row = constp.tile([1, 128], BF16, tag="ones_row", name="ones_row")
            nc.gpsimd.memset(ones_row, 1.0)
            nb_sb = constp.tile([W, 4 * W], BF16, tag="nb", name="nb_sb")
            nc.sync.dma_start(out=nb_sb, in_=negband4[:, :])
            wqkv_sb = []
            for c in range(NCH):
                t = constp.tile([128, 3 * C], BF16, tag=f"wqkv{c}", name=f"wqkv{c}")
                nc.sync.dma_start(out=t, in_=wqkv[c * 128:(c + 1) * 128, :])
                wqkv_sb.append(t)
            wproj_sb = []
            for c in range(NCH):
                t = constp.tile([128, C], BF16, tag=f"wproj{c}", name=f"wproj{c}")
                nc.sync.dma_start(out=t, in_=wproj[c * 128:(c + 1) * 128, :])
                wproj_sb.append(t)
            bias_sb = constp.tile([1, C], BF16, tag="bias", name="bias_sb")
            nc.sync.dma_start(out=bias_sb, in_=bproj[:, :])
            # persistent double-buffered V tiles; the per-head ones column
            # (for the in-matmul softmax denominator) is written exactly once
            v65T = []
            for par in range(2):
                row = []
                for t in range(8):
                    vt = constp.tile(
                        [128, 8 * 65], BF16, tag=f"v65_{par}_{t}",
                        name=f"v65_{par}_{t}",
                    )
                    nc.gpsimd.memset(
                        vt.rearrange("p (h e) -> p h e", e=65)[:, :, 64:65], 1.0
                    )
                    row.append(vt)
                v65T.append(row)

            # all x loads hoisted: every tile written exactly once (no WAR),
            # and DMA prefetch runs maximally ahead
            xin_all = []
            for b in range(BPC):
                xin = []
                for t in range(8):
                    xi = xinp.tile([128, C], BF16, tag="xin", name="xi")
                    nc.sync.dma_start(out=xi, in_=x[b, t * 128:(t + 1) * 128, :])
                    xin.append(xi)
                xin_all.append(xin)

            for b in range(BPC):
                xin = xin_all[b]
                xT = [
                    xtp.tile([128, N], BF16, tag=f"xt{c}", name=f"xt{c}")
                    for c in range(NCH)
                ]
                for c in range(NCH):
                    for half in range(2):
                        tp = pstr.tile([128, 512], BF16, tag="tr", name="tp")
                        for tt in range(4):
                            t = half * 4 + tt
                            nc.tensor.transpose(
                                tp[:, tt * 128:(tt + 1) * 128],
                                xin[t][:, c * 128:(c + 1) * 128],
                                ident,
                            )
                        nc.vector.tensor_copy(
                            xT[c][:, half * 512:(half + 1) * 512], tp
                        )

                # ---- qT / kT [dims, tok] ----
                qT = [
                    qkp.tile([128, N], BF16, tag=f"q{m}", name=f"q{m}")
                    for m in range(4)
                ]
                kT = [
                    qkp.tile([128, N], BF16, tag=f"k{m}", name=f"k{m}")
                    for m in range(4)
                ]
                for m in range(4):
                    for dst, base in ((qT, 0), (kT, C)):
                        for nh in range(2):
                            ps = psp.tile([128, 512], F32, tag="s", name="ps")
                            for c in range(NCH):
                                nc.tensor.matmul(
                                    ps,
                                    wqkv_sb[c][:, base + m * 128: base + (m + 1) * 128],
                                    xT[c][:, nh * 512:(nh + 1) * 512],
                                    start=(c == 0),
                                    stop=(c == NCH - 1),
                                )
                            nc.vector.tensor_copy(
                                dst[m][:, nh * 512:(nh + 1) * 512], ps
                            )

                # ---- V [tok, head-major 64 dims + ones col] ----
                v65 = v65T[b % 2]
                for t in range(8):
                    ps = psp.tile([128, 512], F32, tag="s", name="ps")
                    for c in range(NCH):
                        nc.tensor.matmul(
                            ps,
                            xT[c][:, t * 128:(t + 1) * 128],
                            wqkv_sb[c][:, 2 * C:3 * C],
                            start=(c == 0),
                            stop=(c == NCH - 1),
                        )
                    v3 = v65[t].rearrange("p (h e) -> p h e", e=65)
                    nc.vector.tensor_copy(
                        v3[:, :, 0:64], ps.rearrange("p (h d) -> p h d", d=64)
                    )

                # ---- attention, software-pipelined over heads ----
                oT = [
                    otp.tile([128, N], BF16, tag=f"ot{cc}", name=f"ot{cc}")
                    for cc in range(4)
                ]

                def emit_snb(hd):
                    qt, kt = qT[hd // 2], kT[hd // 2]
                    pb = 64 * (hd % 2)
                    ptiles = [
                        pp.tile([128, N], BF16, tag=f"p{i}", name=f"p{i}")
                        for i in range(8)
                    ]
                    for i in range(8):
                        hlo, hhi = max(0, i - 3), min(7, i + 3)
                        for half in range(2):
                            c0 = max(hlo * 128, half * 512)
                            c1 = min((hhi + 1) * 128, (half + 1) * 512)
                            if c0 >= c1:
                                continue
                            ps = psp.tile([128, 512], F32, tag="s", name="ps")
                            l0, l1 = c0 - half * 512, c1 - half * 512
                            nc.tensor.matmul(
                                ps[:, l0:l1],
                                kt[pb:pb + 64, i * 128:(i + 1) * 128],
                                qt[pb:pb + 64, c0:c1],
                                start=True,
                                stop=False,
                            )
                            nc.tensor.matmul(
                                ps[:, l0:l1],
                                ident,
                                nb_sb[:, 0:c1 - c0],
                                start=False,
                                stop=True,
                            )
                            nc.scalar.activation(
                                ptiles[i][:, c0:c1], ps[:, l0:l1], AF.Exp
                            )
                    return ptiles

                def emit_pv(hd, ptiles):
                    pb = 64 * (hd % 2)
                    for hs in range(2):
                        pv = pspv.tile([65, 512], F32, tag="pv", name="pv")
                        ilist = list(range(max(0, 4 * hs - 3), min(7, 4 * hs + 6) + 1))
                        # widest-coverage block first: its matmul writes the
                        # full 512 cols, resetting the accumulation bank.
                        ilist.sort(
                            key=lambda i: (
                                max(max(0, i - 3), 4 * hs)
                                - min(min(7, i + 3), 4 * hs + 3)
                            )
                        )
                        for bi, i in enumerate(ilist):
                            h0 = max(max(0, i - 3), 4 * hs)
                            h1 = min(min(7, i + 3), 4 * hs + 3)
                            c0, c1 = h0 * 128, (h1 + 1) * 128
                            nc.tensor.matmul(
                                pv[:, c0 - 512 * hs:c1 - 512 * hs],
                                v65[i][:, hd * 65:(hd + 1) * 65],
                                ptiles[i][:, c0:c1],
                                start=(bi == 0),
                                stop=(bi == len(ilist) - 1),
                            )
                        den = smallp.tile([1, 512], BF16, tag="den", name="den")
                        nc.vector.tensor_copy(den, pv[64:65, :])
                        bc = psbc.tile([64, 512], F32, tag="bc", name="bc")
                        nc.tensor.matmul(
                            bc, ones_row[0:1, 0:64], den, start=True, stop=True
                        )
                        rcs = smallp.tile([64, 512], F32, tag="rcs", name="rcs")
                        nc.vector.reciprocal(rcs, bc)
                        nc.vector.tensor_mul(
                            oT[hd // 2][pb:pb + 64, hs * 512:(hs + 1) * 512],
                            pv[0:64, :],
                            rcs,
                        )

                prev = None
                for hd in range(NUM_HEADS):
                    cur = emit_snb(hd)
                    if prev is not None:
                        emit_pv(hd - 1, prev)
                    prev = cur
                emit_pv(NUM_HEADS - 1, prev)

                # ---- proj + bias ----
                osb = osbp.tile([128, 8 * C], F32, tag="osb", name="osb")
                for t in range(8):
                    ps = psp.tile([128, 512], F32, tag="s", name="ps")
                    for cc in range(NCH):
                        nc.tensor.matmul(
                            ps,
                            oT[cc][:, t * 128:(t + 1) * 128],
                            wproj_sb[cc],
                            start=(cc == 0),
                            stop=False,
                        )
                    nc.tensor.matmul(ps, ones_row, bias_sb, start=False, stop=True)
                    if t == 0:
                        # tiny pre-read carries the PE RAW wait so the real
                        # copy keeps only its store-queue WAR wait
                        dmy = smallp.tile([1, 8], F32, tag="dmy", name="dmy")
                        nc.vector.tensor_copy(dmy[0:1, 0:1], ps[0:1, 0:1])
                    nc.vector.tensor_copy(osb[:, t * C:(t + 1) * C], ps)
                out_view = out[b].rearrange("(t p) c -> p t c", p=128)
                nc.gpsimd.dma_start(
                    out=out_view, in_=osb.rearrange("p (t c) -> p t c", c=C)
                )
    _fix_multiwaits(nc)
    return nc


_NC_CACHE = None


def _get_nc():
    global _NC_CACHE
    if _NC_CACHE is None:
        _NC_CACHE = build_nc()
    return _NC_CACHE


def make_in_maps(inputs):
    x = np.asarray(inputs["x"], np.float32)
    w_qkv = np.asarray(inputs["w_qkv"], np.float32).copy()
    w_proj = np.asarray(inputs["w_proj"], np.float32)
    b_proj = np.asarray(inputs["b_proj"], np.float32)
    w_qkv[:, :C] *= SCALE
    bf = ml_dtypes.bfloat16
    inband = np.abs(np.arange(W)[:, None] - np.arange(W)[None, :]) <= WK // 2
    negband = np.where(inband, 0.0, -30.0).astype(np.float32)
    negband4 = np.tile(negband, (1, 4)).astype(bf)
    in_maps = []
    for core in range(NCORES):
        in_maps.append(
            {
                "x": np.ascontiguousarray(x[core * BPC:(core + 1) * BPC]).astype(bf),
                "w_qkv": w_qkv.astype(bf),
                "w_proj": w_proj.astype(bf),
                "b_proj": b_proj.reshape(1, C).astype(bf),
                "negband4": negband4,
            }
        )
    return in_maps


def kernel(**inputs):
    nc = _get_nc()
    in_maps = make_in_maps(inputs)
    res = run_bass_kernel_spmd(nc, in_maps, core_ids=list(range(NCORES)))
    outs = [np.asarray(res.results[i]["out"], np.float32) for i in range(NCORES)]
    return np.concatenate(outs, axis=0)
